# revision 11
# baseline (speedup 1.0000x reference)
"""Trainium2 Bass kernel for nn_BaltNet (2-layer ConvLSTM + decoder + MLP head).

Sharding: data-parallel over batch B=8 (one sample per NeuronCore) for the
recurrent conv part; FC1's [131072, 256] contraction is K-sharded 8 ways
(AllToAll of the decoder features, per-core partial matmul, ReduceScatter).

Layout notes
------------
Conv is computed as matmuls over a zero-padded spatial layout [C, 66, 68]
(1-row halo top/bottom, cols 2..65 interior) so every 3x3 tap is a pure
free-dim offset.  The three vertical taps (ky) are packed into the matmul
contraction dim by keeping row-shifted copies of the input stacked on
partitions; the three horizontal taps (kx) are separate accumulating matmul
passes with shifted column windows.

  A  [105, 66, 68]: layer-0 rhs, 3 groups of (h0[32] + x[3]) at ky=0,-1,+1
      (base group first: engine writes need 32-aligned partition starts)
  Ba [128, 66, 68]: layer-1 rhs, groups (h0+h1)[64] at ky=-1 (p0-63), ky=0
  Bb [ 64, 66, 68]: layer-1 rhs, group  (h0+h1)[64] at ky=+1

Gates: z = [i f o g] on 128 partitions; g-gate weights/bias pre-scaled x2 so
tanh(g) = 2*sigmoid(2g) - 1 and one Sigmoid covers all 128 partitions.

LSTM pointwise runs in a QUARTER-PACKED layout [128, 1024] with partition
p = 32*q + ch (q = 16-row quarter): the i/f/o gates are repacked from the
sigmoid output S [128gates, 4096pix] by 12 small SBUF->SBUF DMA block
copies per step, the g-gate is repacked for free by the tensor_scalar
(2*sig-1) whose OUTPUT is rebased per quarter.  This gives 5 full-width
DVE ops + 1 full-width tanh per step instead of 4x that many [32,1024]
ops, which was the serial critical path of the v1 kernel.
c-state is stored packed persistently; h is unpacked back to the spatial
rhs layout by 4 DMA block copies + full-tensor shifted copies.

Everything 16-bit is fp16 (verified ~1.3e-3 end-to-end vs fp32 reference).
"""

import os
import sys

for _p in ("/opt/trn_rl_repo",):
    if _p not in sys.path and os.path.isdir(_p):
        sys.path.insert(0, _p)

import numpy as np

import concourse.bass as bass
import concourse.mybir as mybir
import concourse.tile as tile
from concourse import bacc
from concourse.bass_utils import run_bass_kernel_spmd

F16 = mybir.dt.float16
F32 = mybir.dt.float32
AF = mybir.ActivationFunctionType
OP = mybir.AluOpType

B, T, C, HID, H, W = 8, 24, 3, 32, 64, 64
G4 = 4 * HID            # 128 gate channels
PH, PW = H + 2, W + 4   # padded spatial: rows 0..65, interior cols 2..65
NPIX = H * W            # 4096
KSL = HID * NPIX // 8   # 16384 per-core FC1 K-slice
N_CORES = 8

TRACE = False           # test.py flips this for profiled runs
_CACHE = {}


def _build_nc():
    nc = bacc.Bacc("TRN2", target_bir_lowering=False, debug=False,
                   num_devices=N_CORES)

    # ---- I/O -------------------------------------------------------------
    xp_d = nc.dram_tensor("xp", [T, C, PH, PW], F16, kind="ExternalInput")
    w0_d = nc.dram_tensor("w0", [105, 3 * G4], F16, kind="ExternalInput")
    w1a_d = nc.dram_tensor("w1a", [128, 3 * G4], F16, kind="ExternalInput")
    w1b_d = nc.dram_tensor("w1b", [64, 3 * G4], F16, kind="ExternalInput")
    wd_d = nc.dram_tensor("wd", [105, 3 * G4], F16, kind="ExternalInput")
    b0_d = nc.dram_tensor("b0", [G4, 1], F32, kind="ExternalInput")
    b1_d = nc.dram_tensor("b1", [G4, 1], F32, kind="ExternalInput")
    bd_d = nc.dram_tensor("bd", [G4, 1], F32, kind="ExternalInput")
    fw_d = nc.dram_tensor("fw", [128, 128 * 256], F16, kind="ExternalInput")
    fb_d = nc.dram_tensor("fb", [128, 2], F32, kind="ExternalInput")
    w2_d = nc.dram_tensor("w2", [128, 2 * 97], F16, kind="ExternalInput")
    b2_d = nc.dram_tensor("b2", [97, 1], F32, kind="ExternalInput")
    out_d = nc.dram_tensor("out", [97, 1], F32, kind="ExternalOutput")

    with tile.TileContext(nc) as tc:
        with (
            tc.tile_pool(name="state", bufs=1) as state,
            tc.tile_pool(name="const", bufs=1) as const,
            tc.tile_pool(name="sgate", bufs=3) as sgate,
            tc.tile_pool(name="pack", bufs=2) as pack,
            tc.tile_pool(name="scr", bufs=2) as scr,
            tc.tile_pool(name="psum", bufs=4, space="PSUM") as psum,
            tc.tile_pool(name="dram", bufs=1, space="DRAM") as dram,
        ):
            # ---- persistent SBUF state ----------------------------------
            A = state.tile([105, PH, PW], F16)    # L0 rhs (h0 + x), 3 ky-groups
            Ba = state.tile([128, PH, PW], F16)   # L1 rhs ky=-1,0
            Bb = state.tile([64, PH, PW], F16)    # L1 rhs ky=+1
            # packed c-state: partition p = 32*q + ch, free = 16 rows x 64 cols
            C0p = state.tile([128, 1024], F16)
            C1p = state.tile([128, 1024], F16)

            # ---- constants ----------------------------------------------
            w0 = const.tile([105, 3 * G4], F16)
            w1a = const.tile([128, 3 * G4], F16)
            w1b = const.tile([64, 3 * G4], F16)
            wd = const.tile([105, 3 * G4], F16)
            b0 = const.tile([G4, 1], F32)
            b1 = const.tile([G4, 1], F32)
            bd = const.tile([G4, 1], F32)
            fw = const.tile([128, 128 * 256], F16)
            fb = const.tile([128, 2], F32)
            w2 = const.tile([128, 2 * 97], F16)
            b2 = const.tile([97, 1], F32)
            ft = const.tile([128, 8, 128], F16)   # A2A result, FC1 lhsT tiles

            # Head ordering: x(0) + L0 weights go FIRST on the sync queue so
            # the first matmuls are not stuck behind the 8.4MB fc1 load.
            nc.gpsimd.memset(A[:], 0.0)           # gpsimd queue: memsets first
            nc.sync.dma_start(out=A[32:35, :, :], in_=xp_d[0])
            nc.sync.dma_start(out=A[67:70, 1:PH, :], in_=xp_d[0, :, 0:PH - 1, :])
            nc.sync.dma_start(out=A[102:105, 0:PH - 1, :],
                              in_=xp_d[0, :, 1:PH, :])
            nc.sync.dma_start(out=w0[:], in_=w0_d[:])
            nc.sync.dma_start(out=b0[:], in_=b0_d[:])
            nc.gpsimd.memset(Ba[:], 0.0)
            nc.gpsimd.memset(Bb[:], 0.0)
            nc.vector.memset(C0p[:], 0.0)
            nc.vector.memset(C1p[:], 0.0)
            for dst, src in ((w1a, w1a_d), (w1b, w1b_d), (wd, wd_d),
                             (b1, b1_d), (bd, bd_d),
                             (fb, fb_d), (w2, w2_d), (b2, b2_d)):
                nc.gpsimd.dma_start(out=dst[:], in_=src[:])
            # big fc1 weight: only needed at the tail; gpsimd queue, split
            for i in range(8):
                sl = slice(i * 4096, (i + 1) * 4096)
                nc.gpsimd.dma_start(out=fw[:, sl], in_=fw_d[:, sl])

            # ---- DRAM bounce buffers for collectives --------------------
            a2a_in = dram.tile([HID, NPIX], F16)
            a2a_out = dram.tile([8, 128, 128], F16)
            z1part = dram.tile([8, 256], F32)
            z1red = dram.tile([256], F32)

            KXS = (-1, 0, 1)

            def conv_step(srcs, bias, Cp, rp_eng):
                """One full ConvLSTM cell step: 4 row-quarters of matmuls +
                sigmoid, gate repack into the quarter-packed layout
                (partition p = 4*ch + q, so each gate repack is ONE DMA whose
                linear stream matches (ch, q, pix) order), then a single
                full-width pointwise.  Returns the packed h tile."""
                npass = len(srcs) * 3
                S = sgate.tile([G4, NPIX], F16, tag="S")
                SP = pack.tile([128, 4, 1024], F16, tag="SP")  # i,f,o,sg
                for rt in range(4):
                    pz = psum.tile([G4, 1024], F32, tag="z", name=f"pz{rt}")
                    ip = 0
                    for buf, K, wt in srcs:
                        for kxi, kx in enumerate(KXS):
                            lhs = wt[:, kxi * G4:(kxi + 1) * G4]
                            for h in range(2):
                                r0 = 16 * rt + 8 * h
                                rhs = buf[0:K, r0 + 1:r0 + 9, 2 + kx:66 + kx]
                                nc.tensor.matmul(
                                    pz[:, 512 * h:512 * h + 512],
                                    lhs, rhs, start=(ip == 0),
                                    stop=(ip == npass - 1))
                            ip += 1
                    sl = slice(rt * 1024, (rt + 1) * 1024)
                    nc.scalar.activation(out=S[:, sl], in_=pz[:],
                                         func=AF.Sigmoid,
                                         bias=bias[:, 0:1], scale=1.0)
                # gate repack: [32ch, 4096] -> [128=(ch,q), 1024], 1 DMA/gate
                for g in range(4):
                    rp_eng[g % len(rp_eng)].dma_start(
                        out=SP[:, g, :], in_=S[32 * g:32 * g + 32, :])
                # full-width pointwise: c' = f*c + i*(2*sg-1); h = o*tanh(c')
                U = scr.tile([128, 2, 1024], F16, tag="U")
                nc.vector.tensor_scalar(
                    out=U[:, 0, :], in0=SP[:, 3, :],
                    scalar1=2.0, scalar2=-1.0, op0=OP.mult, op1=OP.add)
                nc.vector.tensor_mul(U[:, 1, :], SP[:, 0, :], U[:, 0, :])
                nc.vector.tensor_mul(U[:, 0, :], SP[:, 1, :], Cp[:])
                nc.vector.tensor_add(Cp[:], U[:, 0, :], U[:, 1, :])
                TH = scr.tile([128, 1024], F16, tag="TH")
                nc.scalar.activation(out=TH[:], in_=Cp[:], func=AF.Tanh)
                Hp = scr.tile([128, 1024], F16, tag="Hp")
                nc.vector.tensor_mul(Hp[:], SP[:, 2, :], TH[:])
                return Hp

            def h_copies(Hp, dsts, engs):
                """Write packed h [128=(ch,q), 1024] directly into every
                destination group (with its ky row shift) in one DMA each:
                the packed linear stream equals (ch, row, col) order.
                ky=0 -> rows 1:65, ky=-1 -> rows 2:66, ky=+1 -> rows 0:64.
                Halo rows/cols stay zero from init and are never written."""
                for n, (buf, p0, ky) in enumerate(dsts):
                    r0 = 1 - ky
                    engs[n % len(engs)].dma_start(
                        out=buf[p0:p0 + 32, r0:r0 + 64, 2:66], in_=Hp[:])

            # ================= recurrent steps ===========================
            # Layer 1 runs one step behind layer 0 so the PE alternates
            # between the two layers' matmul bursts while the other layer's
            # pointwise chain drains.
            def l1_step():
                Hp1 = conv_step([(Ba, 128, w1a), (Bb, 64, w1b)], b1, C1p,
                                [nc.gpsimd, nc.scalar])
                h_copies(Hp1, [(Ba, 96, 0), (Ba, 32, -1), (Bb, 32, 1)],
                         [nc.gpsimd, nc.scalar, nc.gpsimd])

            def xload(t):
                # x_t into A's 3 ky-groups (ky=0 @32, ky=-1 @67, ky=+1 @102)
                nc.gpsimd.dma_start(out=A[32:35, :, :], in_=xp_d[t])
                nc.gpsimd.dma_start(out=A[67:70, 1:PH, :],
                                    in_=xp_d[t, :, 0:PH - 1, :])
                nc.gpsimd.dma_start(out=A[102:105, 0:PH - 1, :],
                                    in_=xp_d[t, :, 1:PH, :])

            for t in range(T):
                Hp0 = conv_step([(A, 105, w0)], b0, C0p,
                                [nc.sync, nc.scalar])
                # h0(t) into A's 3 ky-groups (next L0 step's rhs)
                h_copies(Hp0, [(A, 0, 0), (A, 35, -1), (A, 70, 1)],
                         [nc.sync, nc.scalar, nc.sync])
                if t + 1 < T:
                    xload(t + 1)       # prefetch; WAR-ordered after L0(t) mms

                if t > 0:
                    l1_step()          # L1(t-1)
                # now h0(t) may overwrite L1's rhs state
                h_copies(Hp0, [(Ba, 64, 0), (Ba, 0, -1), (Bb, 0, 1)],
                         [nc.sync, nc.scalar, nc.sync])

            l1_step()                  # L1(T-1)

            # ================= decoder step ==============================
            # decoder rhs: hT (= L1's last h) into A's 3 ky-groups.  hT sits
            # unpacked at Ba[96:128]; copy with row shifts as before.
            for (p0, ky), eng in zip(((0, 0), (35, -1), (70, 1)),
                                     (nc.sync, nc.scalar, nc.sync)):
                r0 = 1 - ky
                eng.dma_start(out=A[p0:p0 + 32, r0:r0 + 64, 2:66],
                              in_=Ba[96:128, 1:65, 2:66])
            Hpd = conv_step([(A, 105, wd)], bd, C1p,
                            [nc.sync, nc.scalar])

            # ================= FC head ===================================
            # packed (ch,q) stream order == hdc[ch, pix] order: one DMA
            nc.sync.dma_start(out=a2a_in[:], in_=Hpd[:])
            nc.gpsimd.collective_compute(
                "AllToAll", OP.bypass,
                replica_groups=[list(range(N_CORES))],
                ins=[a2a_in[:].opt()], outs=[a2a_out[:].opt()])
            # transposed load with K-index q = p*128 + k2:
            # ft[p, m, k2] = a2a_out[m, p, k2] -- contiguous 128-elem runs
            nc.sync.dma_start(
                out=ft[:],
                in_=a2a_out[:].rearrange("m p k -> p m k"))

            psz = psum.tile([8, 256], F32, tag="z")
            for k2 in range(128):
                nc.tensor.matmul(psz[:], ft[:, :, k2],
                                 fw[:, k2 * 256:(k2 + 1) * 256],
                                 start=(k2 == 0), stop=(k2 == 127))
            z1s = scr.tile([8, 256], F32, tag="z1")
            nc.vector.tensor_copy(z1s[:], psz[:])
            nc.sync.dma_start(out=z1part[:], in_=z1s[:])
            nc.gpsimd.collective_compute(
                "ReduceScatter", OP.add,
                replica_groups=[list(range(N_CORES))],
                ins=[z1part[:].opt()], outs=[z1red[:].opt()])

            zr = scr.tile([128, 2], F32, tag="zr")
            nc.sync.dma_start(out=zr[:],
                              in_=z1red[:].rearrange("(j p) -> p j", p=128))
            zrb = scr.tile([128, 2], F32, tag="zrb")
            nc.vector.tensor_add(zrb[:], zr[:], fb[:])
            h256 = scr.tile([128, 2], F16, tag="h256")
            nc.vector.tensor_scalar_max(h256[:], zrb[:], 0.0)

            ps2 = psum.tile([97, 1], F32, tag="z")
            for j in range(2):
                nc.tensor.matmul(ps2[:], w2[:, j * 97:(j + 1) * 97],
                                 h256[:, j:j + 1],
                                 start=(j == 0), stop=(j == 1))
            outs = scr.tile([97, 1], F32, tag="outs")
            nc.vector.tensor_add(outs[:], ps2[:], b2[:])
            nc.sync.dma_start(out=out_d[:], in_=outs[:])

    nc.compile()
    return nc


def _prep_inputs(x, Wenc0, benc0, Wenc1, benc1, Wdec, bdec,
                 fc1_w, fc1_b, fc2_w, fc2_b):
    """Host-side: pad/reorder/cast everything into device layouts."""
    f16 = np.float16

    def conv_w(Wk, reorder_x):
        # Wk [128, Cin, 3, 3] -> per-kx [ngrp*ch, 128] with ky stacked on
        # partitions; gate-g output channels pre-scaled x2.
        Wk = np.asarray(Wk, np.float32).copy()
        Wk[96:128] *= 2.0
        if reorder_x:  # [x(3), h(32)] -> [h(32), x(3)]
            Wk = np.concatenate([Wk[:, 3:], Wk[:, :3]], axis=1)
        cin = Wk.shape[1]
        out = np.zeros((3 * cin, 3 * G4), np.float32)
        for g, dy in enumerate((1, 0, 2)):   # group order ky = 0, -1, +1
            for kxi in range(3):
                # [cin, 128]
                out[g * cin:(g + 1) * cin, kxi * G4:(kxi + 1) * G4] = \
                    Wk[:, :, dy, kxi].T
        return out.astype(f16)

    def bias_v(b):
        b = np.asarray(b, np.float32).copy()
        b[96:128] *= 2.0
        return b.reshape(G4, 1)

    w0_full = conv_w(Wenc0, True)       # [105, 384]
    wd_full = conv_w(Wdec, True)
    w1_full = conv_w(Wenc1, False)      # [192, 384]; groups ky = 0, -1, +1
    # Ba's partition groups are ky=-1 @0-63, ky=0 @64-127
    w1a = np.ascontiguousarray(
        np.concatenate([w1_full[64:128], w1_full[0:64]], axis=0))
    w1b = np.ascontiguousarray(w1_full[128:192])

    xpad = np.zeros((B, T, C, PH, PW), f16)
    xpad[:, :, :, 1:65, 2:66] = np.asarray(x, np.float32)

    fc1_w = np.asarray(fc1_w, np.float32)
    fb = np.asarray(fc1_b, np.float32).reshape(2, 128).T.copy()  # [128, 2]
    w2 = np.asarray(fc2_w, np.float32).T.reshape(2, 128, 97)
    w2 = np.ascontiguousarray(w2.transpose(1, 0, 2)).reshape(128, 2 * 97)
    b2 = np.asarray(fc2_b, np.float32).reshape(97, 1)

    in_maps = []
    for k in range(N_CORES):
        w1k = fc1_w[:, k * KSL:(k + 1) * KSL].T            # [16384, 256]
        # K-index q = p*128 + k2  ->  fw[p, k2, n] = w1k[p*128 + k2, n]
        fwk = w1k.reshape(128, 128 * 256)
        in_maps.append({
            "xp": np.ascontiguousarray(xpad[k]),
            "w0": w0_full, "w1a": w1a.astype(f16), "w1b": w1b.astype(f16),
            "wd": wd_full,
            "b0": bias_v(benc0), "b1": bias_v(benc1), "bd": bias_v(bdec),
            "fw": fwk.astype(f16), "fb": fb,
            "w2": w2.astype(f16), "b2": b2,
        })
    return in_maps


def kernel(**inputs):
    if "nc" not in _CACHE:
        _CACHE["nc"] = _build_nc()
    nc = _CACHE["nc"]
    in_maps = _prep_inputs(**inputs)
    res = run_bass_kernel_spmd(nc, in_maps, core_ids=list(range(N_CORES)),
                               trace=TRACE)
    _CACHE["last_result"] = res
    out = np.stack([res.results[k]["out"][:, 0] for k in range(N_CORES)])
    return out.astype(np.float32)


# revision 16
# speedup vs baseline: 1.1183x; 1.1183x over previous
"""Trainium2 Bass kernel for nn_BaltNet (2-layer ConvLSTM + decoder + MLP head).

Sharding: data-parallel over batch B=8 (one sample per NeuronCore) for the
recurrent conv part; FC1's [131072, 256] contraction is K-sharded 8 ways
(AllToAll of the decoder features, per-core partial matmul, ReduceScatter).

Layout notes
------------
Conv is computed as matmuls over a zero-padded spatial layout [C, 66, 68]
(1-row halo top/bottom, cols 2..65 interior) so every 3x3 tap is a pure
free-dim offset.  The three vertical taps (ky) are packed into the matmul
contraction dim by keeping row-shifted copies of the input stacked on
partitions; the three horizontal taps (kx) are separate accumulating matmul
passes with shifted column windows.

  A  [105, 66, 68]: layer-0 rhs, 3 groups of (h0[32] + x[3]) at ky=0,-1,+1
      (base group first: engine writes need 32-aligned partition starts)
  Ba [128, 66, 68]: layer-1 rhs, groups (h0+h1)[64] at ky=-1 (p0-63), ky=0
  Bb [ 64, 66, 68]: layer-1 rhs, group  (h0+h1)[64] at ky=+1

Gates: z = [i f o g] on 128 partitions; g-gate weights/bias pre-scaled x2 so
tanh(g) = 2*sigmoid(2g) - 1 and one Sigmoid covers all 128 partitions.

LSTM pointwise runs in a QUARTER-PACKED layout [128, 1024] with partition
p = 32*q + ch (q = 16-row quarter): the i/f/o gates are repacked from the
sigmoid output S [128gates, 4096pix] by 12 small SBUF->SBUF DMA block
copies per step, the g-gate is repacked for free by the tensor_scalar
(2*sig-1) whose OUTPUT is rebased per quarter.  This gives 5 full-width
DVE ops + 1 full-width tanh per step instead of 4x that many [32,1024]
ops, which was the serial critical path of the v1 kernel.
c-state is stored packed persistently; h is unpacked back to the spatial
rhs layout by 4 DMA block copies + full-tensor shifted copies.

Everything 16-bit is fp16 (verified ~1.3e-3 end-to-end vs fp32 reference).
"""

import os
import sys

for _p in ("/opt/trn_rl_repo",):
    if _p not in sys.path and os.path.isdir(_p):
        sys.path.insert(0, _p)

import numpy as np

import concourse.bass as bass
import concourse.mybir as mybir
import concourse.tile as tile
from concourse import bacc
from concourse.bass_utils import run_bass_kernel_spmd

F16 = mybir.dt.float16
F32 = mybir.dt.float32
AF = mybir.ActivationFunctionType
OP = mybir.AluOpType

B, T, C, HID, H, W = 8, 24, 3, 32, 64, 64
G4 = 4 * HID            # 128 gate channels
PH, PW = H + 2, W + 4   # padded spatial: rows 0..65, interior cols 2..65
NPIX = H * W            # 4096
KSL = HID * NPIX // 8   # 16384 per-core FC1 K-slice
N_CORES = 8

TRACE = False           # test.py flips this for profiled runs
_CACHE = {}


def _build_nc():
    nc = bacc.Bacc("TRN2", target_bir_lowering=False, debug=False,
                   num_devices=N_CORES)

    # ---- I/O -------------------------------------------------------------
    xp_d = nc.dram_tensor("xp", [T, C, PH, PW], F16, kind="ExternalInput")
    w0_d = nc.dram_tensor("w0", [105, 3 * G4], F16, kind="ExternalInput")
    w1a_d = nc.dram_tensor("w1a", [128, 3 * G4], F16, kind="ExternalInput")
    w1b_d = nc.dram_tensor("w1b", [64, 3 * G4], F16, kind="ExternalInput")
    wd_d = nc.dram_tensor("wd", [105, 3 * G4], F16, kind="ExternalInput")
    b0_d = nc.dram_tensor("b0", [G4, 1], F32, kind="ExternalInput")
    b1_d = nc.dram_tensor("b1", [G4, 1], F32, kind="ExternalInput")
    bd_d = nc.dram_tensor("bd", [G4, 1], F32, kind="ExternalInput")
    fw_d = nc.dram_tensor("fw", [128, 128 * 256], F16, kind="ExternalInput")
    fb_d = nc.dram_tensor("fb", [128, 2], F32, kind="ExternalInput")
    w2_d = nc.dram_tensor("w2", [128, 2 * 97], F16, kind="ExternalInput")
    b2_d = nc.dram_tensor("b2", [97, 1], F32, kind="ExternalInput")
    out_d = nc.dram_tensor("out", [97, 1], F32, kind="ExternalOutput")

    with tile.TileContext(nc) as tc:
        with (
            tc.tile_pool(name="state", bufs=1) as state,
            tc.tile_pool(name="const", bufs=1) as const,
            tc.tile_pool(name="sgate", bufs=3) as sgate,
            tc.tile_pool(name="pack", bufs=2) as pack,
            tc.tile_pool(name="scr", bufs=2) as scr,
            tc.tile_pool(name="psum", bufs=4, space="PSUM") as psum,
            tc.tile_pool(name="dram", bufs=1, space="DRAM") as dram,
        ):
            # ---- persistent SBUF state ----------------------------------
            A = state.tile([105, PH, PW], F16)    # L0 rhs (h0 + x), 3 ky-groups
            Ba = state.tile([128, PH, PW], F16)   # L1 rhs ky=-1,0
            Bb = state.tile([64, PH, PW], F16)    # L1 rhs ky=+1
            # packed c-state: partition p = 4*ch + q, free = 16 rows x 64 cols
            C0p = state.tile([128, 1024], F16)
            C1p = state.tile([128, 1024], F16)
            # packed h with padded cols so h-copies are row-contiguous
            # (2KB descriptors); halo cols zeroed once and never rewritten
            Hp0s = state.tile([128, 16, PW], F16)
            Hp1s = state.tile([128, 16, PW], F16)

            # ---- constants ----------------------------------------------
            w0 = const.tile([105, 3 * G4], F16)
            w1a = const.tile([128, 3 * G4], F16)
            w1b = const.tile([64, 3 * G4], F16)
            wd = const.tile([105, 3 * G4], F16)
            b0 = const.tile([G4, 1], F32)
            b1 = const.tile([G4, 1], F32)
            bd = const.tile([G4, 1], F32)
            fw = const.tile([128, 128 * 256], F16)
            fb = const.tile([128, 2], F32)
            w2 = const.tile([128, 2 * 97], F16)
            b2 = const.tile([97, 1], F32)
            ft = const.tile([128, 8, 128], F16)   # A2A result, FC1 lhsT tiles

            # Head ordering: x(0) + L0 weights go FIRST on the sync queue so
            # the first matmuls are not stuck behind the 8.4MB fc1 load.
            nc.gpsimd.memset(A[:], 0.0)           # gpsimd queue: memsets first
            nc.sync.dma_start(out=A[32:35, :, :], in_=xp_d[0])
            nc.sync.dma_start(out=A[67:70, 1:PH, :], in_=xp_d[0, :, 0:PH - 1, :])
            nc.sync.dma_start(out=A[102:105, 0:PH - 1, :],
                              in_=xp_d[0, :, 1:PH, :])
            nc.sync.dma_start(out=w0[:], in_=w0_d[:])
            nc.sync.dma_start(out=b0[:], in_=b0_d[:])
            nc.gpsimd.memset(Ba[:], 0.0)
            nc.gpsimd.memset(Bb[:], 0.0)
            nc.vector.memset(C0p[:], 0.0)
            nc.vector.memset(C1p[:], 0.0)
            nc.vector.memset(Hp0s[:], 0.0)
            nc.vector.memset(Hp1s[:], 0.0)
            for dst, src in ((w1a, w1a_d), (w1b, w1b_d), (wd, wd_d),
                             (b1, b1_d), (bd, bd_d),
                             (fb, fb_d), (w2, w2_d), (b2, b2_d)):
                nc.gpsimd.dma_start(out=dst[:], in_=src[:])
            # big fc1 weight: only needed at the tail; gpsimd queue, split
            for i in range(8):
                sl = slice(i * 4096, (i + 1) * 4096)
                nc.gpsimd.dma_start(out=fw[:, sl], in_=fw_d[:, sl])

            # ---- DRAM bounce buffers for collectives --------------------
            a2a_in = dram.tile([HID, NPIX], F16)
            a2a_out = dram.tile([8, 128, 128], F16)
            z1part = dram.tile([8, 256], F32)
            z1red = dram.tile([256], F32)

            KXS = (-1, 0, 1)

            def conv_mms(srcs, bias):
                """Matmul + sigmoid phase of one ConvLSTM step (4 row
                quarters).  Returns the unpacked gate tile S [128g, 4096]."""
                npass = len(srcs) * 3
                S = sgate.tile([G4, NPIX], F16, tag="S")
                for rt in range(4):
                    pz = psum.tile([G4, 1024], F32, tag="z", name=f"pz{rt}")
                    ip = 0
                    for buf, K, wt in srcs:
                        for kxi, kx in enumerate(KXS):
                            lhs = wt[:, kxi * G4:(kxi + 1) * G4]
                            for h in range(2):
                                r0 = 16 * rt + 8 * h
                                rhs = buf[0:K, r0 + 1:r0 + 9, 2 + kx:66 + kx]
                                nc.tensor.matmul(
                                    pz[:, 512 * h:512 * h + 512],
                                    lhs, rhs, start=(ip == 0),
                                    stop=(ip == npass - 1))
                            ip += 1
                    sl = slice(rt * 1024, (rt + 1) * 1024)
                    nc.scalar.activation(out=S[:, sl], in_=pz[:],
                                         func=AF.Sigmoid,
                                         bias=bias[:, 0:1], scale=1.0)
                return S

            def conv_chain(S, Cp, Hps, rp_engs):
                """Gate repack [32ch, 4096] -> [128=(ch,q), 1024] (1 DMA per
                gate; linear streams match) + the full-width pointwise:
                c' = f*c + i*(2*sg-1); h = o*tanh(c') written into the
                padded packed h tile Hps[:, :, 2:66]."""
                SP = pack.tile([128, 4, 1024], F16, tag="SP")  # i,f,o,sg
                for g in range(4):
                    rp_engs[g % len(rp_engs)].dma_start(
                        out=SP[:, g, :], in_=S[32 * g:32 * g + 32, :])
                U = scr.tile([128, 2, 1024], F16, tag="U")
                nc.vector.tensor_scalar(
                    out=U[:, 0, :], in0=SP[:, 3, :],
                    scalar1=2.0, scalar2=-1.0, op0=OP.mult, op1=OP.add)
                nc.vector.tensor_mul(U[:, 1, :], SP[:, 0, :], U[:, 0, :])
                nc.vector.tensor_mul(U[:, 0, :], SP[:, 1, :], Cp[:])
                nc.vector.tensor_add(Cp[:], U[:, 0, :], U[:, 1, :])
                TH = scr.tile([128, 1024], F16, tag="TH")
                nc.scalar.activation(out=TH[:], in_=Cp[:], func=AF.Tanh)
                nc.vector.tensor_mul(Hps[:, :, 2:66], SP[:, 2, :], TH[:])

            def h_copies(Hps, dsts, engs):
                """Write packed padded h [128=(ch,q), 16, 68] directly into
                each destination group (with its ky row shift), one DMA
                each; rows are contiguous on both sides so descriptors are
                ~2KB.  ky=0 -> rows 1:65, ky=-1 -> 2:66, ky=+1 -> 0:64.
                Halo cols come along as zeros; halo rows never written."""
                for n, (buf, p0, ky) in enumerate(dsts):
                    r0 = 1 - ky
                    engs[n % len(engs)].dma_start(
                        out=buf[p0:p0 + 32, r0:r0 + 64, :], in_=Hps[:])

            # ================= recurrent steps ===========================
            # Layer 1 runs one step behind layer 0.  Per iteration both
            # layers' matmul+sigmoid bursts are emitted first (so the Act
            # queue never stalls on a tanh before the other layer's
            # sigmoids), then both pointwise chains + h fan-out copies.
            def xload(t):
                # x_t into A's 3 ky-groups (ky=0 @32, ky=-1 @67, ky=+1 @102)
                nc.gpsimd.dma_start(out=A[32:35, :, :], in_=xp_d[t])
                nc.gpsimd.dma_start(out=A[67:70, 1:PH, :],
                                    in_=xp_d[t, :, 0:PH - 1, :])
                nc.gpsimd.dma_start(out=A[102:105, 0:PH - 1, :],
                                    in_=xp_d[t, :, 1:PH, :])

            L1_SRCS = [(Ba, 128, w1a), (Bb, 64, w1b)]
            for t in range(T):
                S0 = conv_mms([(A, 105, w0)], b0)
                S1 = conv_mms(L1_SRCS, b1) if t > 0 else None  # L1(t-1)

                conv_chain(S0, C0p, Hp0s, [nc.sync, nc.scalar])
                # h0(t) into A's 3 ky-groups (next L0 step's rhs)
                h_copies(Hp0s, [(A, 0, 0), (A, 35, -1), (A, 70, 1)],
                         [nc.sync, nc.scalar, nc.sync])
                if t + 1 < T:
                    xload(t + 1)       # prefetch; WAR-ordered after L0(t) mms
                if S1 is not None:
                    conv_chain(S1, C1p, Hp1s, [nc.gpsimd, nc.scalar])
                    h_copies(Hp1s, [(Ba, 96, 0), (Ba, 32, -1), (Bb, 32, 1)],
                             [nc.gpsimd, nc.gpsimd, nc.gpsimd])
                # h0(t) overwrites L1's rhs state (WAR after L1(t-1) mms)
                h_copies(Hp0s, [(Ba, 64, 0), (Ba, 0, -1), (Bb, 0, 1)],
                         [nc.sync, nc.scalar, nc.sync])

            # L1(T-1): its h feeds only the decoder, directly via A
            S1 = conv_mms(L1_SRCS, b1)
            conv_chain(S1, C1p, Hp1s, [nc.gpsimd, nc.scalar])
            h_copies(Hp1s, [(A, 0, 0), (A, 35, -1), (A, 70, 1)],
                     [nc.sync, nc.scalar, nc.gpsimd])

            # ================= decoder step ==============================
            Sd = conv_mms([(A, 105, wd)], bd)
            conv_chain(Sd, C1p, Hp0s, [nc.sync, nc.scalar])
            Hpd = Hp0s

            # ================= FC head ===================================
            # packed (ch,q) stream order == hdc[ch, pix] order: one DMA
            nc.sync.dma_start(out=a2a_in[:], in_=Hpd[:, :, 2:66])
            nc.gpsimd.collective_compute(
                "AllToAll", OP.bypass,
                replica_groups=[list(range(N_CORES))],
                ins=[a2a_in[:].opt()], outs=[a2a_out[:].opt()])
            # transposed load with K-index q = p*128 + k2:
            # ft[p, m, k2] = a2a_out[m, p, k2] -- contiguous 128-elem runs
            nc.sync.dma_start(
                out=ft[:],
                in_=a2a_out[:].rearrange("m p k -> p m k"))

            psz = psum.tile([8, 256], F32, tag="z")
            for k2 in range(128):
                nc.tensor.matmul(psz[:], ft[:, :, k2],
                                 fw[:, k2 * 256:(k2 + 1) * 256],
                                 start=(k2 == 0), stop=(k2 == 127))
            z1s = scr.tile([8, 256], F32, tag="z1")
            nc.vector.tensor_copy(z1s[:], psz[:])
            nc.sync.dma_start(out=z1part[:], in_=z1s[:])
            nc.gpsimd.collective_compute(
                "ReduceScatter", OP.add,
                replica_groups=[list(range(N_CORES))],
                ins=[z1part[:].opt()], outs=[z1red[:].opt()])

            zr = scr.tile([128, 2], F32, tag="zr")
            nc.sync.dma_start(out=zr[:],
                              in_=z1red[:].rearrange("(j p) -> p j", p=128))
            zrb = scr.tile([128, 2], F32, tag="zrb")
            nc.vector.tensor_add(zrb[:], zr[:], fb[:])
            h256 = scr.tile([128, 2], F16, tag="h256")
            nc.vector.tensor_scalar_max(h256[:], zrb[:], 0.0)

            ps2 = psum.tile([97, 1], F32, tag="z")
            for j in range(2):
                nc.tensor.matmul(ps2[:], w2[:, j * 97:(j + 1) * 97],
                                 h256[:, j:j + 1],
                                 start=(j == 0), stop=(j == 1))
            outs = scr.tile([97, 1], F32, tag="outs")
            nc.vector.tensor_add(outs[:], ps2[:], b2[:])
            nc.sync.dma_start(out=out_d[:], in_=outs[:])

    nc.compile()
    return nc


def _prep_inputs(x, Wenc0, benc0, Wenc1, benc1, Wdec, bdec,
                 fc1_w, fc1_b, fc2_w, fc2_b):
    """Host-side: pad/reorder/cast everything into device layouts."""
    f16 = np.float16

    def conv_w(Wk, reorder_x):
        # Wk [128, Cin, 3, 3] -> per-kx [ngrp*ch, 128] with ky stacked on
        # partitions; gate-g output channels pre-scaled x2.
        Wk = np.asarray(Wk, np.float32).copy()
        Wk[96:128] *= 2.0
        if reorder_x:  # [x(3), h(32)] -> [h(32), x(3)]
            Wk = np.concatenate([Wk[:, 3:], Wk[:, :3]], axis=1)
        cin = Wk.shape[1]
        out = np.zeros((3 * cin, 3 * G4), np.float32)
        for g, dy in enumerate((1, 0, 2)):   # group order ky = 0, -1, +1
            for kxi in range(3):
                # [cin, 128]
                out[g * cin:(g + 1) * cin, kxi * G4:(kxi + 1) * G4] = \
                    Wk[:, :, dy, kxi].T
        return out.astype(f16)

    def bias_v(b):
        b = np.asarray(b, np.float32).copy()
        b[96:128] *= 2.0
        return b.reshape(G4, 1)

    w0_full = conv_w(Wenc0, True)       # [105, 384]
    wd_full = conv_w(Wdec, True)
    w1_full = conv_w(Wenc1, False)      # [192, 384]; groups ky = 0, -1, +1
    # Ba's partition groups are ky=-1 @0-63, ky=0 @64-127
    w1a = np.ascontiguousarray(
        np.concatenate([w1_full[64:128], w1_full[0:64]], axis=0))
    w1b = np.ascontiguousarray(w1_full[128:192])

    xpad = np.zeros((B, T, C, PH, PW), f16)
    xpad[:, :, :, 1:65, 2:66] = np.asarray(x, np.float32)

    fc1_w = np.asarray(fc1_w, np.float32)
    fb = np.asarray(fc1_b, np.float32).reshape(2, 128).T.copy()  # [128, 2]
    w2 = np.asarray(fc2_w, np.float32).T.reshape(2, 128, 97)
    w2 = np.ascontiguousarray(w2.transpose(1, 0, 2)).reshape(128, 2 * 97)
    b2 = np.asarray(fc2_b, np.float32).reshape(97, 1)

    in_maps = []
    for k in range(N_CORES):
        w1k = fc1_w[:, k * KSL:(k + 1) * KSL].T            # [16384, 256]
        # K-index q = p*128 + k2  ->  fw[p, k2, n] = w1k[p*128 + k2, n]
        fwk = w1k.reshape(128, 128 * 256)
        in_maps.append({
            "xp": np.ascontiguousarray(xpad[k]),
            "w0": w0_full, "w1a": w1a.astype(f16), "w1b": w1b.astype(f16),
            "wd": wd_full,
            "b0": bias_v(benc0), "b1": bias_v(benc1), "bd": bias_v(bdec),
            "fw": fwk.astype(f16), "fb": fb,
            "w2": w2.astype(f16), "b2": b2,
        })
    return in_maps


def kernel(**inputs):
    if "nc" not in _CACHE:
        _CACHE["nc"] = _build_nc()
    nc = _CACHE["nc"]
    in_maps = _prep_inputs(**inputs)
    res = run_bass_kernel_spmd(nc, in_maps, core_ids=list(range(N_CORES)),
                               trace=TRACE)
    _CACHE["last_result"] = res
    out = np.stack([res.results[k]["out"][:, 0] for k in range(N_CORES)])
    return out.astype(np.float32)


# revision 19
# speedup vs baseline: 1.1479x; 1.0265x over previous
"""Trainium2 Bass kernel for nn_BaltNet (2-layer ConvLSTM + decoder + MLP head).

Sharding: data-parallel over batch B=8 (one sample per NeuronCore) for the
recurrent conv part; FC1's [131072, 256] contraction is K-sharded 8 ways
(AllToAll of the decoder features, per-core partial matmul, ReduceScatter).

Layout notes
------------
Conv is computed as matmuls over a zero-padded spatial layout [C, 66, 68]
(1-row halo top/bottom, cols 2..65 interior) so every 3x3 tap is a pure
free-dim offset.  The three vertical taps (ky) are packed into the matmul
contraction dim by keeping row-shifted copies of the input stacked on
partitions; the three horizontal taps (kx) are separate accumulating matmul
passes with shifted column windows.

  A  [105, 66, 68]: layer-0 rhs, 3 groups of (h0[32] + x[3]) at ky=0,-1,+1
      (base group first: engine writes need 32-aligned partition starts)
  Ba [128, 66, 68]: layer-1 rhs, groups (h0+h1)[64] at ky=-1 (p0-63), ky=0
  Bb [ 64, 66, 68]: layer-1 rhs, group  (h0+h1)[64] at ky=+1

Gates: z = [i f o g] on 128 partitions; g-gate weights/bias pre-scaled x2 so
tanh(g) = 2*sigmoid(2g) - 1 and one Sigmoid covers all 128 partitions.

LSTM pointwise runs in a QUARTER-PACKED layout [128, 1024] with partition
p = 32*q + ch (q = 16-row quarter): the i/f/o gates are repacked from the
sigmoid output S [128gates, 4096pix] by 12 small SBUF->SBUF DMA block
copies per step, the g-gate is repacked for free by the tensor_scalar
(2*sig-1) whose OUTPUT is rebased per quarter.  This gives 5 full-width
DVE ops + 1 full-width tanh per step instead of 4x that many [32,1024]
ops, which was the serial critical path of the v1 kernel.
c-state is stored packed persistently; h is unpacked back to the spatial
rhs layout by 4 DMA block copies + full-tensor shifted copies.

Everything 16-bit is fp16 (verified ~1.3e-3 end-to-end vs fp32 reference).
"""

import os
import sys

for _p in ("/opt/trn_rl_repo",):
    if _p not in sys.path and os.path.isdir(_p):
        sys.path.insert(0, _p)

import numpy as np

import concourse.bass as bass
import concourse.mybir as mybir
import concourse.tile as tile
from concourse import bacc
from concourse.bass_utils import run_bass_kernel_spmd

F16 = mybir.dt.float16
F32 = mybir.dt.float32
AF = mybir.ActivationFunctionType
OP = mybir.AluOpType

B, T, C, HID, H, W = 8, 24, 3, 32, 64, 64
G4 = 4 * HID            # 128 gate channels
PH, PW = H + 2, W + 4   # padded spatial: rows 0..65, interior cols 2..65
NPIX = H * W            # 4096
KSL = HID * NPIX // 8   # 16384 per-core FC1 K-slice
N_CORES = 8

TRACE = False           # test.py flips this for profiled runs
_CACHE = {}


def _build_nc():
    nc = bacc.Bacc("TRN2", target_bir_lowering=False, debug=False,
                   num_devices=N_CORES)

    # ---- I/O -------------------------------------------------------------
    xp_d = nc.dram_tensor("xp", [T, C, PH, PW], F16, kind="ExternalInput")
    w0_d = nc.dram_tensor("w0", [105, 3 * G4], F16, kind="ExternalInput")
    w1a_d = nc.dram_tensor("w1a", [128, 3 * G4], F16, kind="ExternalInput")
    w1b_d = nc.dram_tensor("w1b", [64, 3 * G4], F16, kind="ExternalInput")
    wd_d = nc.dram_tensor("wd", [105, 3 * G4], F16, kind="ExternalInput")
    b0_d = nc.dram_tensor("b0", [G4, 1], F32, kind="ExternalInput")
    b1_d = nc.dram_tensor("b1", [G4, 1], F32, kind="ExternalInput")
    bd_d = nc.dram_tensor("bd", [G4, 1], F32, kind="ExternalInput")
    fw_d = nc.dram_tensor("fw", [128, 128 * 256], F16, kind="ExternalInput")
    fb_d = nc.dram_tensor("fb", [128, 2], F32, kind="ExternalInput")
    w2_d = nc.dram_tensor("w2", [128, 2 * 97], F16, kind="ExternalInput")
    b2_d = nc.dram_tensor("b2", [97, 1], F32, kind="ExternalInput")
    out_d = nc.dram_tensor("out", [97, 1], F32, kind="ExternalOutput")

    with tile.TileContext(nc) as tc:
        with (
            tc.tile_pool(name="state", bufs=1) as state,
            tc.tile_pool(name="const", bufs=1) as const,
            tc.tile_pool(name="sgate", bufs=3) as sgate,
            tc.tile_pool(name="pack", bufs=2) as pack,
            tc.tile_pool(name="scr", bufs=2) as scr,
            tc.tile_pool(name="psum", bufs=4, space="PSUM") as psum,
            tc.tile_pool(name="dram", bufs=1, space="DRAM") as dram,
        ):
            # ---- persistent SBUF state ----------------------------------
            A = state.tile([105, PH, PW], F16)    # L0 rhs (h0 + x), 3 ky-groups
            Ba = state.tile([128, PH, PW], F16)   # L1 rhs ky=-1,0
            Bb = state.tile([64, PH, PW], F16)    # L1 rhs ky=+1
            # packed c-state: partition p = 4*ch + q, free = 16 rows x 64 cols
            C0p = state.tile([128, 1024], F16)
            C1p = state.tile([128, 1024], F16)
            # packed h with padded cols so h-copies are row-contiguous
            # (2KB descriptors); halo cols zeroed once and never rewritten
            Hp0s = state.tile([128, 16, PW], F16)
            Hp1s = state.tile([128, 16, PW], F16)

            # ---- constants ----------------------------------------------
            w0 = const.tile([105, 3 * G4], F16)
            w1a = const.tile([128, 3 * G4], F16)
            w1b = const.tile([64, 3 * G4], F16)
            wd = const.tile([105, 3 * G4], F16)
            b0 = const.tile([G4, 1], F32)
            b1 = const.tile([G4, 1], F32)
            bd = const.tile([G4, 1], F32)
            fw = const.tile([128, 128 * 256], F16)
            fb = const.tile([128, 2], F32)
            w2 = const.tile([128, 2 * 97], F16)
            b2 = const.tile([97, 1], F32)
            ft = const.tile([128, 8, 128], F16)   # A2A result, FC1 lhsT tiles

            # Head ordering: x(0) + L0 weights go FIRST on the sync queue so
            # the first matmuls are not stuck behind the 8.4MB fc1 load.
            nc.gpsimd.memset(A[:], 0.0)           # gpsimd queue: memsets first
            nc.sync.dma_start(out=A[32:35, :, :], in_=xp_d[0])
            nc.sync.dma_start(out=A[67:70, 1:PH, :], in_=xp_d[0, :, 0:PH - 1, :])
            nc.sync.dma_start(out=A[102:105, 0:PH - 1, :],
                              in_=xp_d[0, :, 1:PH, :])
            nc.sync.dma_start(out=w0[:], in_=w0_d[:])
            nc.sync.dma_start(out=b0[:], in_=b0_d[:])
            nc.gpsimd.memset(Ba[:], 0.0)
            nc.gpsimd.memset(Bb[:], 0.0)
            nc.vector.memset(C0p[:], 0.0)
            nc.vector.memset(C1p[:], 0.0)
            nc.vector.memset(Hp0s[:], 0.0)
            nc.vector.memset(Hp1s[:], 0.0)
            for dst, src in ((w1a, w1a_d), (w1b, w1b_d), (wd, wd_d),
                             (b1, b1_d), (bd, bd_d),
                             (fb, fb_d), (w2, w2_d), (b2, b2_d)):
                nc.gpsimd.dma_start(out=dst[:], in_=src[:])

            # ---- DRAM bounce buffers for collectives --------------------
            a2a_in = dram.tile([HID, NPIX], F16)
            a2a_out = dram.tile([8, 128, 128], F16)
            z1part = dram.tile([8, 256], F32)
            z1red = dram.tile([256], F32)

            KXS = (-1, 0, 1)

            def conv_mms(srcs, bias):
                """Matmul + sigmoid phase of one ConvLSTM step (4 row
                quarters).  Returns the unpacked gate tile S [128g, 4096]."""
                npass = len(srcs) * 3
                S = sgate.tile([G4, NPIX], F16, tag="S")
                for rt in range(4):
                    pz = psum.tile([G4, 1024], F32, tag="z", name=f"pz{rt}")
                    ip = 0
                    for buf, K, wt in srcs:
                        for kxi, kx in enumerate(KXS):
                            lhs = wt[:, kxi * G4:(kxi + 1) * G4]
                            for h in range(2):
                                r0 = 16 * rt + 8 * h
                                rhs = buf[0:K, r0 + 1:r0 + 9, 2 + kx:66 + kx]
                                nc.tensor.matmul(
                                    pz[:, 512 * h:512 * h + 512],
                                    lhs, rhs, start=(ip == 0),
                                    stop=(ip == npass - 1))
                            ip += 1
                    sl = slice(rt * 1024, (rt + 1) * 1024)
                    nc.scalar.activation(out=S[:, sl], in_=pz[:],
                                         func=AF.Sigmoid,
                                         bias=bias[:, 0:1], scale=1.0)
                return S

            def conv_chain(S, Cp, Hps, rp_engs):
                """Gate repack [32ch, 4096] -> [128=(ch,q), 1024] (1 DMA per
                gate; linear streams match) + the full-width pointwise:
                c' = f*c + i*(2*sg-1); h = o*tanh(c') written into the
                padded packed h tile Hps[:, :, 2:66].  Repack order g,i,f,o
                so the DVE chain can start as soon as each gate lands."""
                SP = pack.tile([128, 4, 1024], F16, tag="SP")  # i,f,o,sg
                for n, g in enumerate((3, 0, 1, 2)):
                    rp_engs[n % len(rp_engs)].dma_start(
                        out=SP[:, g, :], in_=S[32 * g:32 * g + 32, :])
                U = scr.tile([128, 2, 1024], F16, tag="U")
                nc.vector.tensor_scalar(
                    out=U[:, 0, :], in0=SP[:, 3, :],
                    scalar1=2.0, scalar2=-1.0, op0=OP.mult, op1=OP.add)
                nc.vector.tensor_mul(U[:, 1, :], SP[:, 0, :], U[:, 0, :])
                nc.vector.tensor_mul(U[:, 0, :], SP[:, 1, :], Cp[:])
                nc.vector.tensor_add(Cp[:], U[:, 0, :], U[:, 1, :])
                TH = scr.tile([128, 1024], F16, tag="TH")
                nc.scalar.activation(out=TH[:], in_=Cp[:], func=AF.Tanh)
                nc.vector.tensor_mul(Hps[:, :, 2:66], SP[:, 2, :], TH[:])

            def h_copies(Hps, dsts, engs):
                """Write packed padded h [128=(ch,q), 16, 68] directly into
                each destination group (with its ky row shift), one DMA
                each; rows are contiguous on both sides so descriptors are
                ~2KB.  ky=0 -> rows 1:65, ky=-1 -> 2:66, ky=+1 -> 0:64.
                Halo cols come along as zeros; halo rows never written."""
                for n, (buf, p0, ky) in enumerate(dsts):
                    r0 = 1 - ky
                    engs[n % len(engs)].dma_start(
                        out=buf[p0:p0 + 32, r0:r0 + 64, :], in_=Hps[:])

            # ================= recurrent steps ===========================
            # Layer 1 runs one step behind layer 0.  Per iteration both
            # layers' matmul+sigmoid bursts are emitted first (so the Act
            # queue never stalls on a tanh before the other layer's
            # sigmoids), then both pointwise chains + h fan-out copies.
            def xload(t):
                # x_t into A's 3 ky-groups (ky=0 @32, ky=-1 @67, ky=+1 @102)
                nc.gpsimd.dma_start(out=A[32:35, :, :], in_=xp_d[t])
                nc.gpsimd.dma_start(out=A[67:70, 1:PH, :],
                                    in_=xp_d[t, :, 0:PH - 1, :])
                nc.gpsimd.dma_start(out=A[102:105, 0:PH - 1, :],
                                    in_=xp_d[t, :, 1:PH, :])

            L1_SRCS = [(Ba, 128, w1a), (Bb, 64, w1b)]
            for t in range(T):
                S0 = conv_mms([(A, 105, w0)], b0)
                S1 = conv_mms(L1_SRCS, b1) if t > 0 else None  # L1(t-1)

                conv_chain(S0, C0p, Hp0s, [nc.sync, nc.gpsimd])
                # h0(t) into A's 3 ky-groups (next L0 step's rhs)
                h_copies(Hp0s, [(A, 0, 0), (A, 35, -1), (A, 70, 1)],
                         [nc.sync, nc.sync, nc.sync])
                if t + 1 < T:
                    xload(t + 1)       # prefetch; WAR-ordered after L0(t) mms
                if 3 <= t < 3 + 16:
                    # trickle in the 8.4MB fc1 weight (needed only at the
                    # tail) so it never contends with the startup loads
                    i = t - 3
                    nc.gpsimd.dma_start(out=fw[:, i * 2048:(i + 1) * 2048],
                                        in_=fw_d[:, i * 2048:(i + 1) * 2048])
                if S1 is not None:
                    conv_chain(S1, C1p, Hp1s, [nc.sync, nc.gpsimd])
                    h_copies(Hp1s, [(Ba, 96, 0), (Ba, 32, -1), (Bb, 32, 1)],
                             [nc.gpsimd, nc.gpsimd, nc.gpsimd])
                # h0(t) overwrites L1's rhs state (WAR after L1(t-1) mms)
                h_copies(Hp0s, [(Ba, 64, 0), (Ba, 0, -1), (Bb, 0, 1)],
                         [nc.sync, nc.sync, nc.sync])

            # L1(T-1): its h feeds only the decoder, directly via A
            S1 = conv_mms(L1_SRCS, b1)
            conv_chain(S1, C1p, Hp1s, [nc.sync, nc.gpsimd])
            h_copies(Hp1s, [(A, 0, 0), (A, 35, -1), (A, 70, 1)],
                     [nc.sync, nc.gpsimd, nc.sync])

            # ================= decoder step ==============================
            Sd = conv_mms([(A, 105, wd)], bd)
            conv_chain(Sd, C1p, Hp0s, [nc.sync, nc.gpsimd])
            Hpd = Hp0s

            # ================= FC head ===================================
            # packed (ch,q) stream order == hdc[ch, pix] order: one DMA
            nc.sync.dma_start(out=a2a_in[:], in_=Hpd[:, :, 2:66])
            nc.gpsimd.collective_compute(
                "AllToAll", OP.bypass,
                replica_groups=[list(range(N_CORES))],
                ins=[a2a_in[:].opt()], outs=[a2a_out[:].opt()])
            # transposed load with K-index q = p*128 + k2:
            # ft[p, m, k2] = a2a_out[m, p, k2] -- contiguous 128-elem runs
            nc.sync.dma_start(
                out=ft[:],
                in_=a2a_out[:].rearrange("m p k -> p m k"))

            psz = psum.tile([8, 256], F32, tag="z")
            for k2 in range(128):
                nc.tensor.matmul(psz[:], ft[:, :, k2],
                                 fw[:, k2 * 256:(k2 + 1) * 256],
                                 start=(k2 == 0), stop=(k2 == 127))
            z1s = scr.tile([8, 256], F32, tag="z1")
            nc.vector.tensor_copy(z1s[:], psz[:])
            nc.sync.dma_start(out=z1part[:], in_=z1s[:])
            nc.gpsimd.collective_compute(
                "ReduceScatter", OP.add,
                replica_groups=[list(range(N_CORES))],
                ins=[z1part[:].opt()], outs=[z1red[:].opt()])

            zr = scr.tile([128, 2], F32, tag="zr")
            nc.sync.dma_start(out=zr[:],
                              in_=z1red[:].rearrange("(j p) -> p j", p=128))
            zrb = scr.tile([128, 2], F32, tag="zrb")
            nc.vector.tensor_add(zrb[:], zr[:], fb[:])
            h256 = scr.tile([128, 2], F16, tag="h256")
            nc.vector.tensor_scalar_max(h256[:], zrb[:], 0.0)

            ps2 = psum.tile([97, 1], F32, tag="z")
            for j in range(2):
                nc.tensor.matmul(ps2[:], w2[:, j * 97:(j + 1) * 97],
                                 h256[:, j:j + 1],
                                 start=(j == 0), stop=(j == 1))
            outs = scr.tile([97, 1], F32, tag="outs")
            nc.vector.tensor_add(outs[:], ps2[:], b2[:])
            nc.sync.dma_start(out=out_d[:], in_=outs[:])

    nc.compile()
    return nc


def _prep_inputs(x, Wenc0, benc0, Wenc1, benc1, Wdec, bdec,
                 fc1_w, fc1_b, fc2_w, fc2_b):
    """Host-side: pad/reorder/cast everything into device layouts."""
    f16 = np.float16

    def conv_w(Wk, reorder_x):
        # Wk [128, Cin, 3, 3] -> per-kx [ngrp*ch, 128] with ky stacked on
        # partitions; gate-g output channels pre-scaled x2.
        Wk = np.asarray(Wk, np.float32).copy()
        Wk[96:128] *= 2.0
        if reorder_x:  # [x(3), h(32)] -> [h(32), x(3)]
            Wk = np.concatenate([Wk[:, 3:], Wk[:, :3]], axis=1)
        cin = Wk.shape[1]
        out = np.zeros((3 * cin, 3 * G4), np.float32)
        for g, dy in enumerate((1, 0, 2)):   # group order ky = 0, -1, +1
            for kxi in range(3):
                # [cin, 128]
                out[g * cin:(g + 1) * cin, kxi * G4:(kxi + 1) * G4] = \
                    Wk[:, :, dy, kxi].T
        return out.astype(f16)

    def bias_v(b):
        b = np.asarray(b, np.float32).copy()
        b[96:128] *= 2.0
        return b.reshape(G4, 1)

    w0_full = conv_w(Wenc0, True)       # [105, 384]
    wd_full = conv_w(Wdec, True)
    w1_full = conv_w(Wenc1, False)      # [192, 384]; groups ky = 0, -1, +1
    # Ba's partition groups are ky=-1 @0-63, ky=0 @64-127
    w1a = np.ascontiguousarray(
        np.concatenate([w1_full[64:128], w1_full[0:64]], axis=0))
    w1b = np.ascontiguousarray(w1_full[128:192])

    xpad = np.zeros((B, T, C, PH, PW), f16)
    xpad[:, :, :, 1:65, 2:66] = np.asarray(x, np.float32)

    fc1_w = np.asarray(fc1_w, np.float32)
    fb = np.asarray(fc1_b, np.float32).reshape(2, 128).T.copy()  # [128, 2]
    w2 = np.asarray(fc2_w, np.float32).T.reshape(2, 128, 97)
    w2 = np.ascontiguousarray(w2.transpose(1, 0, 2)).reshape(128, 2 * 97)
    b2 = np.asarray(fc2_b, np.float32).reshape(97, 1)

    in_maps = []
    for k in range(N_CORES):
        w1k = fc1_w[:, k * KSL:(k + 1) * KSL].T            # [16384, 256]
        # K-index q = p*128 + k2  ->  fw[p, k2, n] = w1k[p*128 + k2, n]
        fwk = w1k.reshape(128, 128 * 256)
        in_maps.append({
            "xp": np.ascontiguousarray(xpad[k]),
            "w0": w0_full, "w1a": w1a.astype(f16), "w1b": w1b.astype(f16),
            "wd": wd_full,
            "b0": bias_v(benc0), "b1": bias_v(benc1), "bd": bias_v(bdec),
            "fw": fwk.astype(f16), "fb": fb,
            "w2": w2.astype(f16), "b2": b2,
        })
    return in_maps


def kernel(**inputs):
    if "nc" not in _CACHE:
        _CACHE["nc"] = _build_nc()
    nc = _CACHE["nc"]
    in_maps = _prep_inputs(**inputs)
    res = run_bass_kernel_spmd(nc, in_maps, core_ids=list(range(N_CORES)),
                               trace=TRACE)
    _CACHE["last_result"] = res
    out = np.stack([res.results[k]["out"][:, 0] for k in range(N_CORES)])
    return out.astype(np.float32)


# revision 24
# speedup vs baseline: 1.4370x; 1.2518x over previous
"""Trainium2 Bass kernel for nn_BaltNet (2-layer ConvLSTM + decoder + MLP head).

Sharding: data-parallel over batch B=8 (one sample per NeuronCore) for the
recurrent conv part; FC1's [131072, 256] contraction is K-sharded 8 ways
(AllToAll of the decoder features, per-core partial matmul, ReduceScatter).

Layout notes
------------
Conv is computed as matmuls over a zero-padded spatial layout [C, 66, 68]
(1-row halo top/bottom, cols 2..65 interior) so every 3x3 tap is a pure
free-dim offset.  The three vertical taps (ky) are packed into the matmul
contraction dim by keeping row-shifted copies of the input stacked on
partitions; the three horizontal taps (kx) are separate accumulating matmul
passes with shifted column windows.

  A  [105, 66, 68]: layer-0 rhs, 3 groups of (h0[32] + x[3]) at ky=0,-1,+1
      (base group first: engine writes need 32-aligned partition starts)
  Ba [128, 66, 68]: layer-1 rhs, groups (h0+h1)[64] at ky=-1 (p0-63), ky=0
  Bb [ 64, 66, 68]: layer-1 rhs, group  (h0+h1)[64] at ky=+1

Gates: z = [i f o g] on 128 partitions; g-gate weights/bias pre-scaled x2 so
tanh(g) = 2*sigmoid(2g) - 1 and one Sigmoid covers all 128 partitions.
Everything 16-bit is fp16 (verified ~1.2e-3 end-to-end vs fp32 reference).
"""

import os
import sys

for _p in ("/opt/trn_rl_repo",):
    if _p not in sys.path and os.path.isdir(_p):
        sys.path.insert(0, _p)

import numpy as np

import concourse.bass as bass
import concourse.mybir as mybir
import concourse.tile as tile
from concourse import bacc
from concourse.bass_utils import run_bass_kernel_spmd

F16 = mybir.dt.float16
F32 = mybir.dt.float32
AF = mybir.ActivationFunctionType
OP = mybir.AluOpType

B, T, C, HID, H, W = 8, 24, 3, 32, 64, 64
G4 = 4 * HID            # 128 gate channels
PH, PW = H + 2, W + 4   # padded spatial: rows 0..65, interior cols 2..65
NPIX = H * W            # 4096
KSL = HID * NPIX // 8   # 16384 per-core FC1 K-slice
N_CORES = 8

TRACE = False           # test.py flips this for profiled runs
_CACHE = {}


def _build_nc():
    nc = bacc.Bacc("TRN2", target_bir_lowering=False, debug=False,
                   num_devices=N_CORES)

    # ---- I/O -------------------------------------------------------------
    xp_d = nc.dram_tensor("xp", [T, C, PH, PW], F16, kind="ExternalInput")
    w0_d = nc.dram_tensor("w0", [105, 3 * G4], F16, kind="ExternalInput")
    w1a_d = nc.dram_tensor("w1a", [128, 3 * G4], F16, kind="ExternalInput")
    w1b_d = nc.dram_tensor("w1b", [64, 3 * G4], F16, kind="ExternalInput")
    wd_d = nc.dram_tensor("wd", [105, 3 * G4], F16, kind="ExternalInput")
    b0_d = nc.dram_tensor("b0", [G4, 1], F32, kind="ExternalInput")
    b1_d = nc.dram_tensor("b1", [G4, 1], F32, kind="ExternalInput")
    bd_d = nc.dram_tensor("bd", [G4, 1], F32, kind="ExternalInput")
    fw_d = nc.dram_tensor("fw", [128, 128 * 256], F16, kind="ExternalInput")
    fb_d = nc.dram_tensor("fb", [128, 2], F32, kind="ExternalInput")
    w2_d = nc.dram_tensor("w2", [128, 2 * 97], F16, kind="ExternalInput")
    b2_d = nc.dram_tensor("b2", [97, 1], F32, kind="ExternalInput")
    out_d = nc.dram_tensor("out", [97, 1], F32, kind="ExternalOutput")

    with tile.TileContext(nc) as tc:
        with (
            tc.tile_pool(name="state", bufs=1) as state,
            tc.tile_pool(name="const", bufs=1) as const,
            tc.tile_pool(name="sgate", bufs=3) as sgate,
            tc.tile_pool(name="scr", bufs=3) as scr,
            tc.tile_pool(name="psum", bufs=4, space="PSUM") as psum,
            tc.tile_pool(name="dram", bufs=1, space="DRAM") as dram,
        ):
            # ---- persistent SBUF state ----------------------------------
            A = state.tile([105, PH, PW], F16)    # L0 rhs (h0 + x), 3 ky-groups
            Ba = state.tile([128, PH, PW], F16)   # L1 rhs ky=-1,0
            Bb = state.tile([64, PH, PW], F16)    # L1 rhs ky=+1
            # c-state lives on partitions 32-63 so TT ops pair with S[32:64]
            cst0 = state.tile([64, NPIX], F16)
            cst1 = state.tile([64, NPIX], F16)
            hdc = state.tile([HID, NPIX], F16)    # decoder h (feat)

            # ---- constants ----------------------------------------------
            w0 = const.tile([105, 3 * G4], F16)
            w1a = const.tile([128, 3 * G4], F16)
            w1b = const.tile([64, 3 * G4], F16)
            wd = const.tile([105, 3 * G4], F16)
            b0 = const.tile([G4, 1], F32)
            b1 = const.tile([G4, 1], F32)
            bd = const.tile([G4, 1], F32)
            fw = const.tile([128, 128 * 256], F16)
            fb = const.tile([128, 2], F32)
            w2 = const.tile([128, 2 * 97], F16)
            b2 = const.tile([97, 1], F32)
            ft = const.tile([128, 8, 128], F16)   # A2A result, FC1 lhsT tiles

            # zero-init state (h=0, c=0, halos=0); A first so xload(0) and
            # the first matmuls start ASAP
            nc.gpsimd.memset(A[:], 0.0)
            nc.sync.dma_start(out=w0[:], in_=w0_d[:])
            nc.sync.dma_start(out=b0[:], in_=b0_d[:])
            nc.gpsimd.memset(Ba[:], 0.0)
            nc.gpsimd.memset(Bb[:], 0.0)
            nc.vector.memset(cst0[:], 0.0)
            nc.vector.memset(cst1[:], 0.0)
            # remaining consts on the gpsimd queue, off the critical path
            for dst, src in ((w1a, w1a_d), (w1b, w1b_d), (wd, wd_d),
                             (b1, b1_d), (bd, bd_d),
                             (fb, fb_d), (w2, w2_d), (b2, b2_d)):
                nc.gpsimd.dma_start(out=dst[:], in_=src[:])
            # the 8.4MB fc1 weight (needed only at the tail) is trickled in
            # during the recurrence -- see the main loop

            # ---- DRAM bounce buffers for collectives --------------------
            a2a_in = dram.tile([HID, NPIX], F16)
            a2a_out = dram.tile([8, 128, 128], F16)
            z1part = dram.tile([8, 256], F32)
            z1red = dram.tile([256], F32)

            KXS = (-1, 0, 1)

            def pointwise_q(S, cst, hdst, rt):
                """LSTM cell update for one quarter (16 image rows).
                TT inputs must share a base partition, so scratch tensors
                are placed at the base of the gate they pair with."""
                sl = slice(rt * 1024, (rt + 1) * 1024)
                # tg = 2*sigmoid(2g) - 1, re-based to partitions 0-31
                tgt = scr.tile([32, 1024], F16, tag="tgt")
                nc.vector.tensor_scalar(
                    out=tgt[:], in0=S[96:128, sl],
                    scalar1=2.0, scalar2=-1.0, op0=OP.mult, op1=OP.add)
                uv = scr.tile([32, 2, 1024], F16, tag="uv")
                nc.vector.tensor_mul(uv[:, 0, :], S[0:32, sl], tgt[:])
                nc.vector.tensor_mul(uv[:, 1, :], S[32:64, sl],
                                     cst[32:64, sl])
                nc.vector.tensor_add(cst[32:64, sl], uv[:, 0, :],
                                     uv[:, 1, :])
                tht = scr.tile([96, 1024], F16, tag="tht")
                nc.scalar.activation(out=tht[64:96, :],
                                     in_=cst[32:64, sl], func=AF.Tanh)
                if hdst is hdc:
                    dst = hdc[:, sl]
                else:
                    buf, p0 = hdst
                    dst = buf[p0:p0 + 32, 1 + 16 * rt:17 + 16 * rt, 2:66]
                nc.vector.tensor_mul(dst, S[64:96, sl], tht[64:96, :])

            def conv_layer(srcs, bias, S, cst, hdst, post_q=None):
                """srcs: list of (tile, K, weights).  Per row-quarter:
                accumulate all passes into a [128,1024] PSUM tile, sigmoid,
                then that quarter's pointwise — so pointwise overlaps the
                next quarter's matmuls.  post_q(rt) is emitted right after
                each quarter's pointwise (e.g. eager collective feeds)."""
                npass = len(srcs) * 3
                for rt in range(4):
                    pz = psum.tile([G4, 1024], F32, tag="z", name=f"pz{rt}")
                    ip = 0
                    for buf, K, wt in srcs:
                        for kxi, kx in enumerate(KXS):
                            lhs = wt[:, kxi * G4:(kxi + 1) * G4]
                            for h in range(2):
                                r0 = 16 * rt + 8 * h
                                rhs = buf[0:K, r0 + 1:r0 + 9, 2 + kx:66 + kx]
                                nc.tensor.matmul(
                                    pz[:, 512 * h:512 * h + 512],
                                    lhs, rhs, start=(ip == 0),
                                    stop=(ip == npass - 1))
                            ip += 1
                    nc.scalar.activation(out=S[:, rt * 1024:(rt + 1) * 1024],
                                         in_=pz[:], func=AF.Sigmoid,
                                         bias=bias[:, 0:1], scale=1.0)
                    pointwise_q(S, cst, hdst, rt)
                    if post_q is not None:
                        post_q(rt)

            def shift_copies(dsts, src, eng):
                """src: (buf, p0) base-group h [32, PH, PW]; dsts: list of
                (buf, p0, ky).  eng picks the HWDGE queue (sync feeds L0's
                rhs, scalar feeds L1's) to avoid head-of-line blocking."""
                sbuf, sp = src
                for buf, p0, ky in dsts:
                    if ky == 0:
                        eng.dma_start(out=buf[p0:p0 + 32, :, :],
                                      in_=sbuf[sp:sp + 32, :, :])
                    elif ky == -1:
                        eng.dma_start(out=buf[p0:p0 + 32, 1:PH, :],
                                      in_=sbuf[sp:sp + 32, 0:PH - 1, :])
                    else:
                        eng.dma_start(out=buf[p0:p0 + 32, 0:PH - 1, :],
                                      in_=sbuf[sp:sp + 32, 1:PH, :])

            # ================= recurrent steps ===========================
            # Layer 1 runs one step behind layer 0 so the PE alternates
            # between the two layers' matmul bursts with no pointwise gap:
            # L1(t-1)'s inputs (h0(t-1), h1(t-2)) are ready before L0(t)
            # even starts.  The h0(t) -> Ba/Bb copies are emitted AFTER
            # L1(t-1) so Tile's program-order dependency keeps them WAR.
            def l1_step():
                S1 = sgate.tile([G4, NPIX], F16, tag="S", name="S1")
                conv_layer([(Ba, 128, w1a), (Bb, 64, w1b)], b1, S1,
                           cst1, (Ba, 96))
                shift_copies([(Ba, 32, -1), (Bb, 32, 1)], (Ba, 96),
                             nc.scalar)

            def xload(t):
                # x_t into A's 3 ky-groups (ky=0 @32, ky=-1 @67, ky=+1 @102)
                nc.sync.dma_start(out=A[32:35, :, :], in_=xp_d[t])
                nc.sync.dma_start(out=A[67:70, 1:PH, :],
                                  in_=xp_d[t, :, 0:PH - 1, :])
                nc.sync.dma_start(out=A[102:105, 0:PH - 1, :],
                                  in_=xp_d[t, :, 1:PH, :])

            xload(0)
            for t in range(T):
                S0 = sgate.tile([G4, NPIX], F16, tag="S", name="S0")
                conv_layer([(A, 105, w0)], b0, S0, cst0, (A, 0))
                if t + 1 < T:
                    xload(t + 1)       # prefetch; WAR-ordered after L0(t) mms
                if 2 <= t < 2 + 16:
                    # trickle in the 8.4MB fc1 weight (tail-only) so it
                    # never contends with the startup or per-step DMAs
                    i = t - 2
                    nc.gpsimd.dma_start(out=fw[:, i * 2048:(i + 1) * 2048],
                                        in_=fw_d[:, i * 2048:(i + 1) * 2048])
                # h0(t) shifted copies within A (next L0 step's rhs)
                shift_copies([(A, 35, -1), (A, 70, 1)], (A, 0), nc.sync)

                if t > 0:
                    l1_step()          # L1(t-1)
                # now h0(t) may overwrite L1's rhs state
                shift_copies([(Ba, 64, 0), (Ba, 0, -1), (Bb, 0, 1)], (A, 0),
                             nc.scalar)

            l1_step()                  # L1(T-1)

            # ================= decoder step ==============================
            shift_copies([(A, 0, 0), (A, 35, -1), (A, 70, 1)], (Ba, 96),
                         nc.sync)
            Sd = sgate.tile([G4, NPIX], F16, tag="S")

            def feed_a2a(rt):
                # stream each decoder quarter into the A2A input as soon as
                # its h lands, so the collective can trigger immediately
                sl = slice(rt * 1024, (rt + 1) * 1024)
                nc.sync.dma_start(out=a2a_in[:, sl], in_=hdc[:, sl])

            conv_layer([(A, 105, wd)], bd, Sd, cst1, hdc, post_q=feed_a2a)

            # ================= FC head ===================================
            nc.gpsimd.collective_compute(
                "AllToAll", OP.bypass,
                replica_groups=[list(range(N_CORES))],
                ins=[a2a_in[:].opt()], outs=[a2a_out[:].opt()])
            # transposed load with K-index q = p*128 + k2:
            # ft[p, m, k2] = a2a_out[m, p, k2] -- contiguous 128-elem runs
            nc.sync.dma_start(
                out=ft[:],
                in_=a2a_out[:].rearrange("m p k -> p m k"))

            psz = psum.tile([8, 256], F32, tag="z")
            for k2 in range(128):
                nc.tensor.matmul(psz[:], ft[:, :, k2],
                                 fw[:, k2 * 256:(k2 + 1) * 256],
                                 start=(k2 == 0), stop=(k2 == 127))
            z1s = scr.tile([8, 256], F32, tag="z1")
            nc.vector.tensor_copy(z1s[:], psz[:])
            nc.sync.dma_start(out=z1part[:], in_=z1s[:])
            nc.gpsimd.collective_compute(
                "ReduceScatter", OP.add,
                replica_groups=[list(range(N_CORES))],
                ins=[z1part[:].opt()], outs=[z1red[:].opt()])

            zr = scr.tile([128, 2], F32, tag="zr")
            nc.sync.dma_start(out=zr[:],
                              in_=z1red[:].rearrange("(j p) -> p j", p=128))
            zrb = scr.tile([128, 2], F32, tag="zrb")
            nc.vector.tensor_add(zrb[:], zr[:], fb[:])
            h256 = scr.tile([128, 2], F16, tag="h256")
            nc.vector.tensor_scalar_max(h256[:], zrb[:], 0.0)

            ps2 = psum.tile([97, 1], F32, tag="z")
            for j in range(2):
                nc.tensor.matmul(ps2[:], w2[:, j * 97:(j + 1) * 97],
                                 h256[:, j:j + 1],
                                 start=(j == 0), stop=(j == 1))
            outs = scr.tile([97, 1], F32, tag="outs")
            nc.vector.tensor_add(outs[:], ps2[:], b2[:])
            nc.sync.dma_start(out=out_d[:], in_=outs[:])

    nc.compile()
    return nc


def _prep_inputs(x, Wenc0, benc0, Wenc1, benc1, Wdec, bdec,
                 fc1_w, fc1_b, fc2_w, fc2_b):
    """Host-side: pad/reorder/cast everything into device layouts."""
    f16 = np.float16

    def conv_w(Wk, reorder_x):
        # Wk [128, Cin, 3, 3] -> per-kx [ngrp*ch, 128] with ky stacked on
        # partitions; gate-g output channels pre-scaled x2.
        Wk = np.asarray(Wk, np.float32).copy()
        Wk[96:128] *= 2.0
        if reorder_x:  # [x(3), h(32)] -> [h(32), x(3)]
            Wk = np.concatenate([Wk[:, 3:], Wk[:, :3]], axis=1)
        cin = Wk.shape[1]
        out = np.zeros((3 * cin, 3 * G4), np.float32)
        for g, dy in enumerate((1, 0, 2)):   # group order ky = 0, -1, +1
            for kxi in range(3):
                # [cin, 128]
                out[g * cin:(g + 1) * cin, kxi * G4:(kxi + 1) * G4] = \
                    Wk[:, :, dy, kxi].T
        return out.astype(f16)

    def bias_v(b):
        b = np.asarray(b, np.float32).copy()
        b[96:128] *= 2.0
        return b.reshape(G4, 1)

    w0_full = conv_w(Wenc0, True)       # [105, 384]
    wd_full = conv_w(Wdec, True)
    w1_full = conv_w(Wenc1, False)      # [192, 384]; groups ky = 0, -1, +1
    # Ba's partition groups are ky=-1 @0-63, ky=0 @64-127
    w1a = np.ascontiguousarray(
        np.concatenate([w1_full[64:128], w1_full[0:64]], axis=0))
    w1b = np.ascontiguousarray(w1_full[128:192])

    xpad = np.zeros((B, T, C, PH, PW), f16)
    xpad[:, :, :, 1:65, 2:66] = np.asarray(x, np.float32)

    fc1_w = np.asarray(fc1_w, np.float32)
    fb = np.asarray(fc1_b, np.float32).reshape(2, 128).T.copy()  # [128, 2]
    w2 = np.asarray(fc2_w, np.float32).T.reshape(2, 128, 97)
    w2 = np.ascontiguousarray(w2.transpose(1, 0, 2)).reshape(128, 2 * 97)
    b2 = np.asarray(fc2_b, np.float32).reshape(97, 1)

    in_maps = []
    for k in range(N_CORES):
        w1k = fc1_w[:, k * KSL:(k + 1) * KSL].T            # [16384, 256]
        # K-index q = p*128 + k2  ->  fw[p, k2, n] = w1k[p*128 + k2, n]
        fwk = w1k.reshape(128, 128 * 256)
        in_maps.append({
            "xp": np.ascontiguousarray(xpad[k]),
            "w0": w0_full, "w1a": w1a.astype(f16), "w1b": w1b.astype(f16),
            "wd": wd_full,
            "b0": bias_v(benc0), "b1": bias_v(benc1), "bd": bias_v(bdec),
            "fw": fwk.astype(f16), "fb": fb,
            "w2": w2.astype(f16), "b2": b2,
        })
    return in_maps


def kernel(**inputs):
    if "nc" not in _CACHE:
        _CACHE["nc"] = _build_nc()
    nc = _CACHE["nc"]
    in_maps = _prep_inputs(**inputs)
    res = run_bass_kernel_spmd(nc, in_maps, core_ids=list(range(N_CORES)),
                               trace=TRACE)
    _CACHE["last_result"] = res
    out = np.stack([res.results[k]["out"][:, 0] for k in range(N_CORES)])
    return out.astype(np.float32)

